# revision 27
# baseline (speedup 1.0000x reference)
"""Trainium2 Bass kernel for fused QKV linear + multi-adapter LoRA (moe_routing).

Reference computation (all fp32):
    base = x @ W^T + bias                      x:[B,S,D]  W:[3D,D]
    tmp[p,n,b,s,r]  = x . lora_A[p,n,r,:]      (down-projection, rank 16)
    tmp *= scaling[n] * lora_masks[n,b]
    lora[p,b,s,o]   = tmp . lora_B[p,n,o,r]    (up-projection, summed over n)
    out = base + concat_p(lora)                [B,S,3D]

Strategy: row-parallel over the flattened (B*S) dimension — each of the 8
cores computes 1024 rows x all 12288 output columns.  Each core's rows
live inside ONE batch, so its LoRA contribution is a fixed low-rank
update; the host merges it into the weights per batch
    W'_b = W + sum_n scaling[n]*mask[n,b] * concat_p(B_pn @ A_pn)
(~1.6 GFLOP/batch in numpy, exact in fp32) and the device runs a pure
GEMM: out = x @ W'^T + bias.  This removes the LoRA down/up projections
from the PE entirely (-4% cycles); HW probes show the PE weight loads
fully overlap with the moving stream, so device time is the pure
moving-column roofline 6144 matmuls x 512 cycles = 3.15 Mcycles/core
(1311 us at the 2.4 GHz nameplate clock; the shared axon chips run at
~1.9-2.2 GHz depending on neighbor-tenant power state, and can throttle
~2x for seconds at a time).

Device layout (per core, bf16 matmuls with fp32 PSUM accumulation):
    xk  [128, 32, 1024]    x^T tiles: [k%128, k//128, m]
    wk  [96, 128, 32, 128] W'^T tiles per output tile: [ot, k%128, k//128, o]
    bias[128, 96]          bias[ot*128+op] at [op, ot]
    out [96, 128, 1024]    out^T tiles: [ot, o, m]

Per output tile ot (96): 32 k-tiles x 2 m-chunks of N=512 matmuls
accumulate into PSUM [128, 1024]; a DVE tensor_scalar add applies bias
while evacuating PSUM -> SBUF bf16; DMA out.  x streams in over the
gpsimd+scalar DMA queues in k-tile order (first tiles split in half);
W' streams on the sync queue, prefetched wpool-deep (first tiles
chunked so the first Ldweights is not gated on a full 1 MiB DMA).  The
first two output tiles are interleaved over the x stream so the PE has
4 matmuls of work per arriving k-tile.  A post-trace pass drops
Ldweights that reload an identical stationary (fewer PE instructions;
HW probes show loads overlap with compute either way).  fp8 was probed
and rejected: DoubleRow gives 2x FLOPs via doubled contraction, so the
3-pass hi/lo-split GEMM the 2e-2 gate requires costs 1.5x bf16.  bf16
everywhere gives rel err ~2.6e-3 vs the fp32 reference (gate: 2e-2).
"""

import numpy as np
import ml_dtypes
from contextlib import ExitStack

import concourse.bass as bass
import concourse.tile as tile
from concourse import bacc, mybir, inst_simplify
from concourse.bass_utils import run_bass_kernel_spmd

BF16 = ml_dtypes.bfloat16

B, S, D = 4, 2048, 4096
OUT = 3 * D
N_CORES = 8
M = B * S                 # 8192 flattened rows
MC = M // N_CORES         # 1024 rows per core
P = 128
KT = D // P               # 32 k-tiles
OT = OUT // P             # 96 output tiles
MM_N = 512                # moving-operand width per matmul
N_MCHUNK = MC // MM_N     # 2

_CACHE: dict = {}
EVAC_ENGINE = "vector"    # PSUM->SBUF evacuation engine (vector=DVE)


def _dedupe_ldweights(nc) -> int:
    """Remove back-to-back redundant InstLdweights.

    bacc's move_matmul_waits_to_ldweights splits every InstMatmult into an
    InstLdweights + a non-self-loading InstMatmult.  Consecutive matmuls
    sharing a stationary operand then carry redundant reloads; drop an
    InstLdweights when the PE stream since the previous load has only
    Matmults/sem-waits and the load signature (memref/offset/access
    pattern/dtype/tile geometry) is identical.  Waits on a dropped load
    migrate to the next InstMatmult; generate_event_semaphores runs
    afterwards and re-legalizes wait counts.  (HW probes show Ldweights
    overlap with compute anyway — this just trims PE instruction count.)
    """
    removed = 0
    for blk in nc.m.functions[0].blocks:
        insts = list(blk.instructions)
        last_sig = None
        pending_waits = []
        keep = []
        for inst in insts:
            tn = type(inst).__name__
            if tn == "InstLdweights":
                ap = inst.ins[0]
                sig = (ap.memref, ap.offset, str(ap.ap), str(ap.dtype),
                       str(getattr(inst, "tile_position", None)),
                       str(getattr(inst, "tile_size", None)),
                       str(getattr(inst, "perf_mode", None)),
                       str(getattr(inst, "is_transpose", None)))
                if sig == last_sig:
                    si = inst.sync_info
                    assert not (si and si.on_update), \
                        "dropping Ldweights with on_update"
                    if si and si.on_wait:
                        pending_waits.extend(si.on_wait)
                    removed += 1
                    continue
                last_sig = sig
            elif tn == "InstMatmult":
                if pending_waits:
                    si = inst.sync_info
                    if si is None:
                        inst.sync_info = mybir.SyncInfo(
                            on_wait=list(pending_waits), on_update=[])
                    else:
                        si.on_wait = list(si.on_wait) + pending_waits
                    pending_waits = []
            elif getattr(inst, "engine", None) == mybir.EngineType.PE and \
                    tn != "InstEventSemaphore":
                last_sig = None
            keep.append(inst)
        assert not pending_waits
        if len(keep) != len(insts):
            del blk.instructions[:]
            for i in keep:
                blk.instructions.append(i)
    return removed


def _compile(nc):
    """bacc.Bacc.compile() with the Ldweights dedupe injected right after
    the matmul-split pass (same pass order as bacc.py)."""
    nc.insert_bir_kernel_barrier_sem_inc()
    nc.move_matmul_waits_to_ldweights()
    _dedupe_ldweights(nc)
    nc.generate_event_semaphores()
    nc.remove_dead_instructions_after_branch()
    nc.validate_blocks()
    nc.dce_regs()
    nc.thread_jumps()
    nc.remove_dead_blocks()
    nc.remove_dead_allocations()
    nc.verify_switch_hints()
    nc.alloc_regs()
    inst_simplify.simplify(nc)
    nc.fuse_regops()
    nc.fuse_blocks()
    nc.replace_nops_with_events()
    for engine in nc.engines:
        nc.fuse_nops(engine)
    nc.remove_dead_nops()
    nc.remove_dangling_data()
    nc.generate_event_semaphores()
    nc.insert_library_loads()
    nc.insert_act_table_loads()
    nc.insert_hostgen_rebases()
    nc.codegen_inst_isa_subclasses()


def _build(loop_iters: int | None = None):
    """Trace + compile the per-core Bass program (same program on all cores).

    loop_iters: if set, wrap the body in a hardware For loop that executes
    it that many times per dispatch (used only for benchmarking)."""
    fp32 = mybir.dt.float32
    bf16 = mybir.dt.bfloat16

    nc = bacc.Bacc("TRN2", target_bir_lowering=False, debug=False,
                   num_devices=N_CORES)
    xk = nc.dram_tensor("xk", [P, KT, MC], bf16, kind="ExternalInput").ap()
    wk = nc.dram_tensor("wk", [OT, P, KT, P], bf16, kind="ExternalInput").ap()
    bias = nc.dram_tensor("bias", [P, OT], fp32, kind="ExternalInput").ap()
    # Output staged as bf16: halves the out-DMA traffic (48 -> 24 MiB/core)
    # and doubles DVE evacuation throughput; the host casts back to fp32.
    out = nc.dram_tensor("out", [OT, P, MC], bf16, kind="ExternalOutput").ap()

    with tile.TileContext(nc) as tc, ExitStack() as ctx:
        const = ctx.enter_context(tc.tile_pool(name="const", bufs=1))
        wpool = ctx.enter_context(tc.tile_pool(name="wpool", bufs=9))
        opool = ctx.enter_context(tc.tile_pool(name="opool", bufs=6))
        pspool = ctx.enter_context(tc.tile_pool(name="pspool", bufs=4, space="PSUM"))

        loop_cm = tc.For_i(0, loop_iters, 1) if loop_iters else None
        if loop_cm is not None:
            loop_cm.__enter__()
        try:
            # x streams in over both free DMA queues (gpsimd SWDGE + scalar
            # HWDGE) in k-tile (= consumption) order; the first k-tiles are
            # split in half so the first matmuls start after 128 KiB, not
            # 256.  The sync HWDGE ring is left free for the W' stream.
            xsb = const.tile([P, KT, MC], bf16, name="xsb")
            engs = [nc.gpsimd, nc.scalar]
            for kt in range(KT):
                if kt < 4:
                    for h in range(2):
                        hsl = slice(h * MM_N, (h + 1) * MM_N)
                        engs[h].dma_start(xsb[:, kt, hsl], xk[:, kt, hsl])
                else:
                    engs[kt % 2].dma_start(xsb[:, kt, :], xk[:, kt, :])
            biassb = const.tile([P, OT], fp32, name="biassb")
            nc.gpsimd.dma_start(biassb, bias)

            # The head is paced by the x stream (one k-tile feeds only
            # 2x512 matmul cycles per output tile): interleave the first
            # PIPE output tiles so the PE has PIPE*2 matmuls per arriving
            # k-tile, and chunk their W-tile DMAs so the first Ldweights
            # is gated on 8 k-slices (256 KiB), not the full 1 MiB tile.
            PIPE = 2
            wsbs = []
            for ot in range(PIPE):
                wsb = wpool.tile([P, KT, P], bf16, name="wsb")
                nchunk = 8 if ot == 0 else 4
                for j in range(nchunk):
                    ksl = slice(j * (KT // nchunk), (j + 1) * (KT // nchunk))
                    nc.sync.dma_start(wsb[:, ksl, :], wk[ot, :, ksl, :])
                wsbs.append(wsb)
            pss = [pspool.tile([P, MC], fp32, name="ps") for _ in range(PIPE)]
            for kt in range(KT):
                for i in range(PIPE):
                    for mc_i in range(N_MCHUNK):
                        msl = slice(mc_i * MM_N, (mc_i + 1) * MM_N)
                        nc.tensor.matmul(pss[i][:, msl],
                                         lhsT=wsbs[i][:, kt, :],
                                         rhs=xsb[:, kt, msl],
                                         start=(kt == 0),
                                         stop=(kt == KT - 1))
            def evac(osb, ps, biascol):
                # PSUM -> SBUF bf16 with bias add.  DVE (vector) by default;
                # "scalar" routes it to the Activation engine instead (its
                # own SBUF port pair - probes whether DVE evacs starve the
                # gpsimd SWDGE x-stream via the shared-port lock).
                if EVAC_ENGINE == "scalar":
                    nc.scalar.add(osb, ps, biascol)
                else:
                    nc.vector.tensor_scalar_add(osb, ps, biascol)

            for i in range(PIPE):
                osb = opool.tile([P, MC], bf16, name="osb")
                evac(osb, pss[i], biassb[:, i:i + 1])
                nc.scalar.dma_start(out[i], osb)

            # Steady state: x fully resident, W' prefetched wpool-deep.
            for ot in range(PIPE, OT):
                wsb = wpool.tile([P, KT, P], bf16, name="wsb")
                if ot < PIPE + 2:
                    for j in range(4):
                        ksl = slice(j * (KT // 4), (j + 1) * (KT // 4))
                        nc.sync.dma_start(wsb[:, ksl, :], wk[ot, :, ksl, :])
                else:
                    nc.sync.dma_start(wsb, wk[ot])
                ps = pspool.tile([P, MC], fp32, name="ps")
                for kt in range(KT):
                    for mc_i in range(N_MCHUNK):
                        msl = slice(mc_i * MM_N, (mc_i + 1) * MM_N)
                        nc.tensor.matmul(ps[:, msl], lhsT=wsb[:, kt, :],
                                         rhs=xsb[:, kt, msl],
                                         start=(kt == 0),
                                         stop=(kt == KT - 1))
                osb = opool.tile([P, MC], bf16, name="osb")
                evac(osb, ps, biassb[:, ot:ot + 1])
                nc.scalar.dma_start(out[ot], osb)
        finally:
            if loop_cm is not None:
                loop_cm.__exit__(None, None, None)

    _compile(nc)
    return nc


def get_nc(loop_iters: int | None = None):
    key = ("nc", loop_iters)
    if key not in _CACHE:
        _CACHE[key] = _build(loop_iters)
    return _CACHE[key]


def prep_in_maps(inputs: dict) -> list[dict]:
    """Merge LoRA into per-batch weights, shard + retile into 8 core maps."""
    x = np.asarray(inputs["x"], np.float32).reshape(M, D)
    w = np.asarray(inputs["weight"], np.float32)
    bias = np.asarray(inputs["bias"], np.float32)
    lora_A = np.asarray(inputs["lora_A"], np.float32)   # [3, n, R, D]
    lora_B = np.asarray(inputs["lora_B"], np.float32)   # [3, n, D, R]
    scaling = np.asarray(inputs["scaling"], np.float32)
    masks = np.asarray(inputs["lora_masks"], np.float32)

    wmat = scaling[:, None] * masks                     # [n, B]
    biasd = np.ascontiguousarray(bias.reshape(OT, P).T)

    wk_by_batch: dict[int, np.ndarray] = {}

    def wk_for_batch(b_idx: int) -> np.ndarray:
        if b_idx not in wk_by_batch:
            wb = w.reshape(3, D, D).copy()              # [p, o, d]
            for n in np.nonzero(wmat[:, b_idx])[0]:
                s = wmat[n, b_idx]
                for p in range(3):
                    wb[p] += s * (lora_B[p, n] @ lora_A[p, n])
            wk_by_batch[b_idx] = np.ascontiguousarray(
                wb.reshape(OT, P, KT, P).transpose(0, 3, 2, 1)).astype(BF16)
        return wk_by_batch[b_idx]

    in_maps = []
    for c in range(N_CORES):
        xs = x[c * MC:(c + 1) * MC]                     # [MC, D]
        xkc = np.ascontiguousarray(
            xs.reshape(MC, KT, P).transpose(2, 1, 0)).astype(BF16)
        b_idx = (c * MC) // S                           # batch of these rows
        in_maps.append({"xk": xkc, "wk": wk_for_batch(b_idx), "bias": biasd})
    return in_maps


def run_device(in_maps: list[dict]):
    nc = get_nc()
    return run_bass_kernel_spmd(nc, in_maps, core_ids=list(range(N_CORES)))


def assemble(results: list[dict]) -> np.ndarray:
    big = np.empty((M, OUT), np.float32)
    for c in range(N_CORES):
        big[c * MC:(c + 1) * MC] = \
            results[c]["out"].reshape(OUT, MC).T.astype(np.float32)
    return big.reshape(B, S, OUT)


def kernel(**inputs) -> np.ndarray:
    in_maps = prep_in_maps(inputs)
    res = run_device(in_maps)
    return assemble(res.results)


# revision 29
# speedup vs baseline: 1.0115x; 1.0115x over previous
"""Trainium2 Bass kernel for fused QKV linear + multi-adapter LoRA (moe_routing).

Reference computation (all fp32):
    base = x @ W^T + bias                      x:[B,S,D]  W:[3D,D]
    tmp[p,n,b,s,r]  = x . lora_A[p,n,r,:]      (down-projection, rank 16)
    tmp *= scaling[n] * lora_masks[n,b]
    lora[p,b,s,o]   = tmp . lora_B[p,n,o,r]    (up-projection, summed over n)
    out = base + concat_p(lora)                [B,S,3D]

Strategy: row-parallel over the flattened (B*S) dimension — each of the 8
cores computes 1024 rows x all 12288 output columns.  Each core's rows
live inside ONE batch, so its LoRA contribution is a fixed low-rank
update; the host merges it into the weights per batch
    W'_b = W + sum_n scaling[n]*mask[n,b] * concat_p(B_pn @ A_pn)
(~1.6 GFLOP/batch in numpy, exact in fp32) and the device runs a pure
GEMM: out = x @ W'^T + bias.  This removes the LoRA down/up projections
from the PE entirely (-4% cycles); HW probes show the PE weight loads
fully overlap with the moving stream, so device time is the pure
moving-column roofline 6144 matmuls x 512 cycles = 3.15 Mcycles/core
(1311 us at the 2.4 GHz nameplate clock; the shared axon chips run at
~1.9-2.2 GHz depending on neighbor-tenant power state, and can throttle
~2x for seconds at a time).

Device layout (per core, bf16 matmuls with fp32 PSUM accumulation):
    xk  [128, 32, 1024]    x^T tiles: [k%128, k//128, m]
    wk  [96, 128, 32, 128] W'^T tiles per output tile: [ot, k%128, k//128, o]
    bias[128, 96]          bias[ot*128+op] at [op, ot]
    out [96, 128, 1024]    out^T tiles: [ot, o, m]

Per output tile ot (96): 32 k-tiles x 2 m-chunks of N=512 matmuls
accumulate into PSUM [128, 1024]; a DVE tensor_scalar add applies bias
while evacuating PSUM -> SBUF bf16; DMA out.  x streams in over the
gpsimd+scalar DMA queues in k-tile order (first tiles split in half);
W' streams on the sync queue, prefetched wpool-deep (first tiles
chunked so the first Ldweights is not gated on a full 1 MiB DMA).  The
first two output tiles are interleaved over the x stream so the PE has
4 matmuls of work per arriving k-tile.  A post-trace pass drops
Ldweights that reload an identical stationary (fewer PE instructions;
HW probes show loads overlap with compute either way).  fp8 was probed
and rejected: DoubleRow gives 2x FLOPs via doubled contraction, so the
3-pass hi/lo-split GEMM the 2e-2 gate requires costs 1.5x bf16.  bf16
everywhere gives rel err ~2.6e-3 vs the fp32 reference (gate: 2e-2).
"""

import numpy as np
import ml_dtypes
from contextlib import ExitStack

import concourse.bass as bass
import concourse.tile as tile
from concourse import bacc, mybir, inst_simplify
from concourse.bass_utils import run_bass_kernel_spmd

BF16 = ml_dtypes.bfloat16

B, S, D = 4, 2048, 4096
OUT = 3 * D
N_CORES = 8
M = B * S                 # 8192 flattened rows
MC = M // N_CORES         # 1024 rows per core
P = 128
KT = D // P               # 32 k-tiles
OT = OUT // P             # 96 output tiles
MM_N = 512                # moving-operand width per matmul
N_MCHUNK = MC // MM_N     # 2

_CACHE: dict = {}
EVAC_ENGINE = "vector"    # PSUM->SBUF evacuation engine (vector=DVE)


def _dedupe_ldweights(nc) -> int:
    """Remove back-to-back redundant InstLdweights.

    bacc's move_matmul_waits_to_ldweights splits every InstMatmult into an
    InstLdweights + a non-self-loading InstMatmult.  Consecutive matmuls
    sharing a stationary operand then carry redundant reloads; drop an
    InstLdweights when the PE stream since the previous load has only
    Matmults/sem-waits and the load signature (memref/offset/access
    pattern/dtype/tile geometry) is identical.  Waits on a dropped load
    migrate to the next InstMatmult; generate_event_semaphores runs
    afterwards and re-legalizes wait counts.  (HW probes show Ldweights
    overlap with compute anyway — this just trims PE instruction count.)
    """
    removed = 0
    for blk in nc.m.functions[0].blocks:
        insts = list(blk.instructions)
        last_sig = None
        pending_waits = []
        keep = []
        for inst in insts:
            tn = type(inst).__name__
            if tn == "InstLdweights":
                ap = inst.ins[0]
                sig = (ap.memref, ap.offset, str(ap.ap), str(ap.dtype),
                       str(getattr(inst, "tile_position", None)),
                       str(getattr(inst, "tile_size", None)),
                       str(getattr(inst, "perf_mode", None)),
                       str(getattr(inst, "is_transpose", None)))
                if sig == last_sig:
                    si = inst.sync_info
                    assert not (si and si.on_update), \
                        "dropping Ldweights with on_update"
                    if si and si.on_wait:
                        pending_waits.extend(si.on_wait)
                    removed += 1
                    continue
                last_sig = sig
            elif tn == "InstMatmult":
                if pending_waits:
                    si = inst.sync_info
                    if si is None:
                        inst.sync_info = mybir.SyncInfo(
                            on_wait=list(pending_waits), on_update=[])
                    else:
                        si.on_wait = list(si.on_wait) + pending_waits
                    pending_waits = []
            elif getattr(inst, "engine", None) == mybir.EngineType.PE and \
                    tn != "InstEventSemaphore":
                last_sig = None
            keep.append(inst)
        assert not pending_waits
        if len(keep) != len(insts):
            del blk.instructions[:]
            for i in keep:
                blk.instructions.append(i)
    return removed


def _compile(nc):
    """bacc.Bacc.compile() with the Ldweights dedupe injected right after
    the matmul-split pass (same pass order as bacc.py)."""
    nc.insert_bir_kernel_barrier_sem_inc()
    nc.move_matmul_waits_to_ldweights()
    _dedupe_ldweights(nc)
    nc.generate_event_semaphores()
    nc.remove_dead_instructions_after_branch()
    nc.validate_blocks()
    nc.dce_regs()
    nc.thread_jumps()
    nc.remove_dead_blocks()
    nc.remove_dead_allocations()
    nc.verify_switch_hints()
    nc.alloc_regs()
    inst_simplify.simplify(nc)
    nc.fuse_regops()
    nc.fuse_blocks()
    nc.replace_nops_with_events()
    for engine in nc.engines:
        nc.fuse_nops(engine)
    nc.remove_dead_nops()
    nc.remove_dangling_data()
    nc.generate_event_semaphores()
    nc.insert_library_loads()
    nc.insert_act_table_loads()
    nc.insert_hostgen_rebases()
    nc.codegen_inst_isa_subclasses()


def _build(loop_iters: int | None = None):
    """Trace + compile the per-core Bass program (same program on all cores).

    loop_iters: if set, wrap the body in a hardware For loop that executes
    it that many times per dispatch (used only for benchmarking)."""
    fp32 = mybir.dt.float32
    bf16 = mybir.dt.bfloat16

    nc = bacc.Bacc("TRN2", target_bir_lowering=False, debug=False,
                   num_devices=N_CORES)
    xk = nc.dram_tensor("xk", [P, KT, MC], bf16, kind="ExternalInput").ap()
    wk = nc.dram_tensor("wk", [OT, P, KT, P], bf16, kind="ExternalInput").ap()
    bias = nc.dram_tensor("bias", [P, OT], fp32, kind="ExternalInput").ap()
    # Output staged as bf16: halves the out-DMA traffic (48 -> 24 MiB/core)
    # and doubles DVE evacuation throughput; the host casts back to fp32.
    out = nc.dram_tensor("out", [OT, P, MC], bf16, kind="ExternalOutput").ap()

    with tile.TileContext(nc) as tc, ExitStack() as ctx:
        const = ctx.enter_context(tc.tile_pool(name="const", bufs=1))
        wpool = ctx.enter_context(tc.tile_pool(name="wpool", bufs=9))
        opool = ctx.enter_context(tc.tile_pool(name="opool", bufs=6))
        pspool = ctx.enter_context(tc.tile_pool(name="pspool", bufs=4, space="PSUM"))

        loop_cm = tc.For_i(0, loop_iters, 1) if loop_iters else None
        if loop_cm is not None:
            loop_cm.__enter__()
        try:
            # x streams in over both free DMA queues (gpsimd SWDGE + scalar
            # HWDGE) in k-tile (= consumption) order; the first k-tiles are
            # split in half so the first matmuls start after 128 KiB, not
            # 256.  The sync HWDGE ring is left free for the W' stream.
            xsb = const.tile([P, KT, MC], bf16, name="xsb")
            # The very first chunk the PE needs (kt0, m 0:512) rides the
            # fast-starting sync HWDGE queue ahead of the W chunks; the
            # gpsimd SWDGE queue takes ~us to produce its first descriptors.
            nc.sync.dma_start(xsb[:, 0, 0:MM_N], xk[:, 0, 0:MM_N])
            engs = [nc.scalar, nc.gpsimd]
            for kt in range(KT):
                if kt == 0:
                    nc.scalar.dma_start(xsb[:, 0, MM_N:MC], xk[:, 0, MM_N:MC])
                elif kt < 4:
                    for h in range(2):
                        hsl = slice(h * MM_N, (h + 1) * MM_N)
                        engs[h].dma_start(xsb[:, kt, hsl], xk[:, kt, hsl])
                else:
                    engs[kt % 2].dma_start(xsb[:, kt, :], xk[:, kt, :])
            biassb = const.tile([P, OT], fp32, name="biassb")
            nc.gpsimd.dma_start(biassb, bias)

            # The head is paced by the x stream (one k-tile feeds only
            # 2x512 matmul cycles per output tile): interleave the first
            # PIPE output tiles so the PE has PIPE*2 matmuls per arriving
            # k-tile, and chunk their W-tile DMAs so the first Ldweights
            # is gated on 8 k-slices (256 KiB), not the full 1 MiB tile.
            PIPE = 2
            # The head interleaves ot0/ot1 per k-tile, so their W chunks
            # must interleave on the sync queue too (all-of-ot0 first would
            # stall ot1's first matmul ~3us behind a 1 MiB transfer).  The
            # first chunks are small so the first Ldweights gates on 64 KiB.
            wsbs = [wpool.tile([P, KT, P], bf16, name="wsb")
                    for _ in range(PIPE)]
            for ksl in (slice(0, 2), slice(2, 4), slice(4, 8),
                        slice(8, 16), slice(16, 24), slice(24, 32)):
                for i in range(PIPE):
                    nc.sync.dma_start(wsbs[i][:, ksl, :], wk[i, :, ksl, :])
            pss = [pspool.tile([P, MC], fp32, name="ps") for _ in range(PIPE)]
            for kt in range(KT):
                for i in range(PIPE):
                    for mc_i in range(N_MCHUNK):
                        msl = slice(mc_i * MM_N, (mc_i + 1) * MM_N)
                        nc.tensor.matmul(pss[i][:, msl],
                                         lhsT=wsbs[i][:, kt, :],
                                         rhs=xsb[:, kt, msl],
                                         start=(kt == 0),
                                         stop=(kt == KT - 1))
            def evac(osb, ps, biascol):
                # PSUM -> SBUF bf16 with bias add.  DVE (vector) by default;
                # "scalar" routes it to the Activation engine instead (its
                # own SBUF port pair - probes whether DVE evacs starve the
                # gpsimd SWDGE x-stream via the shared-port lock).
                if EVAC_ENGINE == "scalar":
                    nc.scalar.add(osb, ps, biascol)
                else:
                    nc.vector.tensor_scalar_add(osb, ps, biascol)

            for i in range(PIPE):
                osb = opool.tile([P, MC], bf16, name="osb")
                evac(osb, pss[i], biassb[:, i:i + 1])
                nc.scalar.dma_start(out[i], osb)

            # Steady state: x fully resident, W' prefetched wpool-deep.
            for ot in range(PIPE, OT):
                wsb = wpool.tile([P, KT, P], bf16, name="wsb")
                if ot < PIPE + 2:
                    for j in range(4):
                        ksl = slice(j * (KT // 4), (j + 1) * (KT // 4))
                        nc.sync.dma_start(wsb[:, ksl, :], wk[ot, :, ksl, :])
                else:
                    nc.sync.dma_start(wsb, wk[ot])
                ps = pspool.tile([P, MC], fp32, name="ps")
                for kt in range(KT):
                    for mc_i in range(N_MCHUNK):
                        msl = slice(mc_i * MM_N, (mc_i + 1) * MM_N)
                        nc.tensor.matmul(ps[:, msl], lhsT=wsb[:, kt, :],
                                         rhs=xsb[:, kt, msl],
                                         start=(kt == 0),
                                         stop=(kt == KT - 1))
                osb = opool.tile([P, MC], bf16, name="osb")
                evac(osb, ps, biassb[:, ot:ot + 1])
                nc.scalar.dma_start(out[ot], osb)
        finally:
            if loop_cm is not None:
                loop_cm.__exit__(None, None, None)

    _compile(nc)
    return nc


def get_nc(loop_iters: int | None = None):
    key = ("nc", loop_iters)
    if key not in _CACHE:
        _CACHE[key] = _build(loop_iters)
    return _CACHE[key]


def prep_in_maps(inputs: dict) -> list[dict]:
    """Merge LoRA into per-batch weights, shard + retile into 8 core maps."""
    x = np.asarray(inputs["x"], np.float32).reshape(M, D)
    w = np.asarray(inputs["weight"], np.float32)
    bias = np.asarray(inputs["bias"], np.float32)
    lora_A = np.asarray(inputs["lora_A"], np.float32)   # [3, n, R, D]
    lora_B = np.asarray(inputs["lora_B"], np.float32)   # [3, n, D, R]
    scaling = np.asarray(inputs["scaling"], np.float32)
    masks = np.asarray(inputs["lora_masks"], np.float32)

    wmat = scaling[:, None] * masks                     # [n, B]
    biasd = np.ascontiguousarray(bias.reshape(OT, P).T)

    wk_by_batch: dict[int, np.ndarray] = {}

    def wk_for_batch(b_idx: int) -> np.ndarray:
        if b_idx not in wk_by_batch:
            wb = w.reshape(3, D, D).copy()              # [p, o, d]
            for n in np.nonzero(wmat[:, b_idx])[0]:
                s = wmat[n, b_idx]
                for p in range(3):
                    wb[p] += s * (lora_B[p, n] @ lora_A[p, n])
            wk_by_batch[b_idx] = np.ascontiguousarray(
                wb.reshape(OT, P, KT, P).transpose(0, 3, 2, 1)).astype(BF16)
        return wk_by_batch[b_idx]

    in_maps = []
    for c in range(N_CORES):
        xs = x[c * MC:(c + 1) * MC]                     # [MC, D]
        xkc = np.ascontiguousarray(
            xs.reshape(MC, KT, P).transpose(2, 1, 0)).astype(BF16)
        b_idx = (c * MC) // S                           # batch of these rows
        in_maps.append({"xk": xkc, "wk": wk_for_batch(b_idx), "bias": biasd})
    return in_maps


def run_device(in_maps: list[dict]):
    nc = get_nc()
    return run_bass_kernel_spmd(nc, in_maps, core_ids=list(range(N_CORES)))


def assemble(results: list[dict]) -> np.ndarray:
    big = np.empty((M, OUT), np.float32)
    for c in range(N_CORES):
        big[c * MC:(c + 1) * MC] = \
            results[c]["out"].reshape(OUT, MC).T.astype(np.float32)
    return big.reshape(B, S, OUT)


def kernel(**inputs) -> np.ndarray:
    in_maps = prep_in_maps(inputs)
    res = run_device(in_maps)
    return assemble(res.results)


# revision 30
# speedup vs baseline: 1.0316x; 1.0199x over previous
"""Trainium2 Bass kernel for fused QKV linear + multi-adapter LoRA (moe_routing).

Reference computation (all fp32):
    base = x @ W^T + bias                      x:[B,S,D]  W:[3D,D]
    tmp[p,n,b,s,r]  = x . lora_A[p,n,r,:]      (down-projection, rank 16)
    tmp *= scaling[n] * lora_masks[n,b]
    lora[p,b,s,o]   = tmp . lora_B[p,n,o,r]    (up-projection, summed over n)
    out = base + concat_p(lora)                [B,S,3D]

Strategy: row-parallel over the flattened (B*S) dimension — each of the 8
cores computes 1024 rows x all 12288 output columns.  Each core's rows
live inside ONE batch, so its LoRA contribution is a fixed low-rank
update; the host merges it into the weights per batch
    W'_b = W + sum_n scaling[n]*mask[n,b] * concat_p(B_pn @ A_pn)
(~1.6 GFLOP/batch in numpy, exact in fp32) and the device runs a pure
GEMM: out = x @ W'^T + bias.  This removes the LoRA down/up projections
from the PE entirely (-4% cycles); HW probes show the PE weight loads
fully overlap with the moving stream, so device time is the pure
moving-column roofline 6144 matmuls x 512 cycles = 3.15 Mcycles/core
(1311 us at the 2.4 GHz nameplate clock; the shared axon chips run at
~1.9-2.2 GHz depending on neighbor-tenant power state, and can throttle
~2x for seconds at a time).

Device layout (per core, bf16 matmuls with fp32 PSUM accumulation):
    xk  [128, 32, 1024]    x^T tiles: [k%128, k//128, m]
    wk  [96, 128, 32, 128] W'^T tiles per output tile: [ot, k%128, k//128, o]
    bias[128, 96]          bias[ot*128+op] at [op, ot]
    out [96, 128, 1024]    out^T tiles: [ot, o, m]

Per output tile ot (96): 32 k-tiles x 2 m-chunks of N=512 matmuls
accumulate into PSUM [128, 1024]; a DVE tensor_scalar add applies bias
while evacuating PSUM -> SBUF bf16; DMA out.  x streams in over the
gpsimd+scalar DMA queues in k-tile order (first tiles split in half);
W' streams on the sync queue, prefetched wpool-deep (first tiles
chunked so the first Ldweights is not gated on a full 1 MiB DMA).  The
first two output tiles are interleaved over the x stream so the PE has
4 matmuls of work per arriving k-tile.  A post-trace pass drops
Ldweights that reload an identical stationary (fewer PE instructions;
HW probes show loads overlap with compute either way).  fp8 was probed
and rejected: DoubleRow gives 2x FLOPs via doubled contraction, so the
3-pass hi/lo-split GEMM the 2e-2 gate requires costs 1.5x bf16.  bf16
everywhere gives rel err ~2.6e-3 vs the fp32 reference (gate: 2e-2).
"""

import numpy as np
import ml_dtypes
from contextlib import ExitStack

import concourse.bass as bass
import concourse.tile as tile
from concourse import bacc, mybir, inst_simplify
from concourse.bass_utils import run_bass_kernel_spmd

BF16 = ml_dtypes.bfloat16

B, S, D = 4, 2048, 4096
OUT = 3 * D
N_CORES = 8
M = B * S                 # 8192 flattened rows
MC = M // N_CORES         # 1024 rows per core
P = 128
KT = D // P               # 32 k-tiles
OT = OUT // P             # 96 output tiles
MM_N = 512                # moving-operand width per matmul
N_MCHUNK = MC // MM_N     # 2

_CACHE: dict = {}
EVAC_ENGINE = "vector"    # PSUM->SBUF evacuation engine (vector=DVE)


def _dedupe_ldweights(nc) -> int:
    """Remove back-to-back redundant InstLdweights.

    bacc's move_matmul_waits_to_ldweights splits every InstMatmult into an
    InstLdweights + a non-self-loading InstMatmult.  Consecutive matmuls
    sharing a stationary operand then carry redundant reloads; drop an
    InstLdweights when the PE stream since the previous load has only
    Matmults/sem-waits and the load signature (memref/offset/access
    pattern/dtype/tile geometry) is identical.  Waits on a dropped load
    migrate to the next InstMatmult; generate_event_semaphores runs
    afterwards and re-legalizes wait counts.  (HW probes show Ldweights
    overlap with compute anyway — this just trims PE instruction count.)
    """
    removed = 0
    for blk in nc.m.functions[0].blocks:
        insts = list(blk.instructions)
        last_sig = None
        pending_waits = []
        keep = []
        for inst in insts:
            tn = type(inst).__name__
            if tn == "InstLdweights":
                ap = inst.ins[0]
                sig = (ap.memref, ap.offset, str(ap.ap), str(ap.dtype),
                       str(getattr(inst, "tile_position", None)),
                       str(getattr(inst, "tile_size", None)),
                       str(getattr(inst, "perf_mode", None)),
                       str(getattr(inst, "is_transpose", None)))
                if sig == last_sig:
                    si = inst.sync_info
                    assert not (si and si.on_update), \
                        "dropping Ldweights with on_update"
                    if si and si.on_wait:
                        pending_waits.extend(si.on_wait)
                    removed += 1
                    continue
                last_sig = sig
            elif tn == "InstMatmult":
                if pending_waits:
                    si = inst.sync_info
                    if si is None:
                        inst.sync_info = mybir.SyncInfo(
                            on_wait=list(pending_waits), on_update=[])
                    else:
                        si.on_wait = list(si.on_wait) + pending_waits
                    pending_waits = []
            elif getattr(inst, "engine", None) == mybir.EngineType.PE and \
                    tn != "InstEventSemaphore":
                last_sig = None
            keep.append(inst)
        assert not pending_waits
        if len(keep) != len(insts):
            del blk.instructions[:]
            for i in keep:
                blk.instructions.append(i)
    return removed


def _compile(nc):
    """bacc.Bacc.compile() with the Ldweights dedupe injected right after
    the matmul-split pass (same pass order as bacc.py)."""
    nc.insert_bir_kernel_barrier_sem_inc()
    nc.move_matmul_waits_to_ldweights()
    _dedupe_ldweights(nc)
    nc.generate_event_semaphores()
    nc.remove_dead_instructions_after_branch()
    nc.validate_blocks()
    nc.dce_regs()
    nc.thread_jumps()
    nc.remove_dead_blocks()
    nc.remove_dead_allocations()
    nc.verify_switch_hints()
    nc.alloc_regs()
    inst_simplify.simplify(nc)
    nc.fuse_regops()
    nc.fuse_blocks()
    nc.replace_nops_with_events()
    for engine in nc.engines:
        nc.fuse_nops(engine)
    nc.remove_dead_nops()
    nc.remove_dangling_data()
    nc.generate_event_semaphores()
    nc.insert_library_loads()
    nc.insert_act_table_loads()
    nc.insert_hostgen_rebases()
    nc.codegen_inst_isa_subclasses()


def _build(loop_iters: int | None = None):
    """Trace + compile the per-core Bass program (same program on all cores).

    loop_iters: if set, wrap the body in a hardware For loop that executes
    it that many times per dispatch (used only for benchmarking)."""
    fp32 = mybir.dt.float32
    bf16 = mybir.dt.bfloat16

    nc = bacc.Bacc("TRN2", target_bir_lowering=False, debug=False,
                   num_devices=N_CORES)
    xk = nc.dram_tensor("xk", [P, KT, MC], bf16, kind="ExternalInput").ap()
    wk = nc.dram_tensor("wk", [OT, P, KT, P], bf16, kind="ExternalInput").ap()
    bias = nc.dram_tensor("bias", [P, OT], fp32, kind="ExternalInput").ap()
    # Output staged as bf16: halves the out-DMA traffic (48 -> 24 MiB/core)
    # and doubles DVE evacuation throughput; the host casts back to fp32.
    out = nc.dram_tensor("out", [OT, P, MC], bf16, kind="ExternalOutput").ap()

    with tile.TileContext(nc) as tc, ExitStack() as ctx:
        const = ctx.enter_context(tc.tile_pool(name="const", bufs=1))
        wpool = ctx.enter_context(tc.tile_pool(name="wpool", bufs=12))
        opool = ctx.enter_context(tc.tile_pool(name="opool", bufs=6))
        pspool = ctx.enter_context(tc.tile_pool(name="pspool", bufs=4, space="PSUM"))

        loop_cm = tc.For_i(0, loop_iters, 1) if loop_iters else None
        if loop_cm is not None:
            loop_cm.__enter__()
        try:
            # x streams in over both free DMA queues (gpsimd SWDGE + scalar
            # HWDGE) in k-tile (= consumption) order; the first k-tiles are
            # split in half so the first matmuls start after 128 KiB, not
            # 256.  The sync HWDGE ring is left free for the W' stream.
            xsb = const.tile([P, KT, MC], bf16, name="xsb")
            # The very first chunk the PE needs (kt0, m 0:512) rides the
            # fast-starting sync HWDGE queue ahead of the W chunks; the
            # gpsimd SWDGE queue takes ~us to produce its first descriptors.
            nc.sync.dma_start(xsb[:, 0, 0:MM_N], xk[:, 0, 0:MM_N])
            engs = [nc.scalar, nc.gpsimd]
            for kt in range(KT):
                if kt == 0:
                    nc.scalar.dma_start(xsb[:, 0, MM_N:MC], xk[:, 0, MM_N:MC])
                elif kt < 4:
                    for h in range(2):
                        hsl = slice(h * MM_N, (h + 1) * MM_N)
                        engs[h].dma_start(xsb[:, kt, hsl], xk[:, kt, hsl])
                else:
                    engs[kt % 2].dma_start(xsb[:, kt, :], xk[:, kt, :])
            biassb = const.tile([P, OT], fp32, name="biassb")
            nc.gpsimd.dma_start(biassb, bias)

            # The head is paced by the x stream (one k-tile feeds only
            # 2x512 matmul cycles per output tile): interleave the first
            # PIPE output tiles so the PE has PIPE*2 matmuls per arriving
            # k-tile, and chunk their W-tile DMAs so the first Ldweights
            # is gated on 8 k-slices (256 KiB), not the full 1 MiB tile.
            PIPE = 2
            # The head interleaves ot0/ot1 per k-tile, so their W chunks
            # must interleave on the sync queue too (all-of-ot0 first would
            # stall ot1's first matmul ~3us behind a 1 MiB transfer).  The
            # first chunks are small so the first Ldweights gates on 64 KiB.
            wsbs = [wpool.tile([P, KT, P], bf16, name="wsb")
                    for _ in range(PIPE)]
            for ksl in (slice(0, 2), slice(2, 4), slice(4, 8),
                        slice(8, 16), slice(16, 24), slice(24, 32)):
                for i in range(PIPE):
                    nc.sync.dma_start(wsbs[i][:, ksl, :], wk[i, :, ksl, :])
            pss = [pspool.tile([P, MC], fp32, name="ps") for _ in range(PIPE)]
            for kt in range(KT):
                for i in range(PIPE):
                    for mc_i in range(N_MCHUNK):
                        msl = slice(mc_i * MM_N, (mc_i + 1) * MM_N)
                        nc.tensor.matmul(pss[i][:, msl],
                                         lhsT=wsbs[i][:, kt, :],
                                         rhs=xsb[:, kt, msl],
                                         start=(kt == 0),
                                         stop=(kt == KT - 1))
            def evac(osb, ps, biascol):
                # PSUM -> SBUF bf16 with bias add.  DVE (vector) by default;
                # "scalar" routes it to the Activation engine instead (its
                # own SBUF port pair - probes whether DVE evacs starve the
                # gpsimd SWDGE x-stream via the shared-port lock).
                if EVAC_ENGINE == "scalar":
                    nc.scalar.add(osb, ps, biascol)
                else:
                    nc.vector.tensor_scalar_add(osb, ps, biascol)

            for i in range(PIPE):
                osb = opool.tile([P, MC], bf16, name="osb")
                evac(osb, pss[i], biassb[:, i:i + 1])
                nc.scalar.dma_start(out[i], osb)

            # Steady state: x fully resident, W' prefetched wpool-deep.
            for ot in range(PIPE, OT):
                wsb = wpool.tile([P, KT, P], bf16, name="wsb")
                if ot < PIPE + 2:
                    for j in range(4):
                        ksl = slice(j * (KT // 4), (j + 1) * (KT // 4))
                        nc.sync.dma_start(wsb[:, ksl, :], wk[ot, :, ksl, :])
                else:
                    nc.sync.dma_start(wsb, wk[ot])
                ps = pspool.tile([P, MC], fp32, name="ps")
                for kt in range(KT):
                    for mc_i in range(N_MCHUNK):
                        msl = slice(mc_i * MM_N, (mc_i + 1) * MM_N)
                        nc.tensor.matmul(ps[:, msl], lhsT=wsb[:, kt, :],
                                         rhs=xsb[:, kt, msl],
                                         start=(kt == 0),
                                         stop=(kt == KT - 1))
                osb = opool.tile([P, MC], bf16, name="osb")
                evac(osb, ps, biassb[:, ot:ot + 1])
                nc.scalar.dma_start(out[ot], osb)
        finally:
            if loop_cm is not None:
                loop_cm.__exit__(None, None, None)

    _compile(nc)
    return nc


def get_nc(loop_iters: int | None = None):
    key = ("nc", loop_iters)
    if key not in _CACHE:
        _CACHE[key] = _build(loop_iters)
    return _CACHE[key]


def prep_in_maps(inputs: dict) -> list[dict]:
    """Merge LoRA into per-batch weights, shard + retile into 8 core maps."""
    x = np.asarray(inputs["x"], np.float32).reshape(M, D)
    w = np.asarray(inputs["weight"], np.float32)
    bias = np.asarray(inputs["bias"], np.float32)
    lora_A = np.asarray(inputs["lora_A"], np.float32)   # [3, n, R, D]
    lora_B = np.asarray(inputs["lora_B"], np.float32)   # [3, n, D, R]
    scaling = np.asarray(inputs["scaling"], np.float32)
    masks = np.asarray(inputs["lora_masks"], np.float32)

    wmat = scaling[:, None] * masks                     # [n, B]
    biasd = np.ascontiguousarray(bias.reshape(OT, P).T)

    wk_by_batch: dict[int, np.ndarray] = {}

    def wk_for_batch(b_idx: int) -> np.ndarray:
        if b_idx not in wk_by_batch:
            wb = w.reshape(3, D, D).copy()              # [p, o, d]
            for n in np.nonzero(wmat[:, b_idx])[0]:
                s = wmat[n, b_idx]
                for p in range(3):
                    wb[p] += s * (lora_B[p, n] @ lora_A[p, n])
            wk_by_batch[b_idx] = np.ascontiguousarray(
                wb.reshape(OT, P, KT, P).transpose(0, 3, 2, 1)).astype(BF16)
        return wk_by_batch[b_idx]

    in_maps = []
    for c in range(N_CORES):
        xs = x[c * MC:(c + 1) * MC]                     # [MC, D]
        xkc = np.ascontiguousarray(
            xs.reshape(MC, KT, P).transpose(2, 1, 0)).astype(BF16)
        b_idx = (c * MC) // S                           # batch of these rows
        in_maps.append({"xk": xkc, "wk": wk_for_batch(b_idx), "bias": biasd})
    return in_maps


def run_device(in_maps: list[dict]):
    nc = get_nc()
    return run_bass_kernel_spmd(nc, in_maps, core_ids=list(range(N_CORES)))


def assemble(results: list[dict]) -> np.ndarray:
    big = np.empty((M, OUT), np.float32)
    for c in range(N_CORES):
        big[c * MC:(c + 1) * MC] = \
            results[c]["out"].reshape(OUT, MC).T.astype(np.float32)
    return big.reshape(B, S, OUT)


def kernel(**inputs) -> np.ndarray:
    in_maps = prep_in_maps(inputs)
    res = run_device(in_maps)
    return assemble(res.results)
